# revision 1
# baseline (speedup 1.0000x reference)
"""CGCNN (gnn_message_passing) Trainium2 kernel — 8-core SPMD.

Strategy:
  - Nodes partitioned contiguously across 8 cores (6250/core, padded to 6272);
    edges assigned to the core owning their dst node, sorted by dst, grouped
    into 128-edge chunks that never cross a 128-node dst block (host padding,
    pad edges read an all-zero table row so they contribute exactly 0).
  - Per conv layer each core computes projection tables
      A_src = v @ [Wm_src|Ws_src]  (AllGathered; gathered per edge by src via
                                    dma_gather over 4 SWDGE queues)
      A_dst = v @ [Wm_dst|Ws_dst]  (local DRAM; gathered per edge by dst)
    z[e] = A_src[src] + A_dst[dst] + ef[e] @ Wef  (ef-projection via matmul of
    pre-transposed edge features; adds are group-wide vector ops).
  - BatchNorm is exact: pass 1 spills z to DRAM and accumulates sum/sumsq in
    on-chip accumulators (pads are exact zeros), tiny AllReduce; pass 2
    reloads z, applies folded BN affine + sigmoid/softplus (built from
    Exp/Ln/Abs/Relu so one activation table serves the whole kernel) and
    scatter-sums h into the local agg block via per-chunk indicator matmuls
    (dst-block index read into a register for the dynamic accumulate).
  - Node BN: local sums + tiny AllReduce. Readout (per-graph mean + 2 MLPs +
    head) computed redundantly per core via graph-indicator matmuls + one
    small AllReduce.  Linear biases feeding BN cancel and are ignored.
"""

import sys
import os
from contextlib import ExitStack

sys.path.insert(0, "/opt/trn_rl_repo")

import numpy as np

import concourse.bass as bass
import concourse.bacc as bacc
import concourse.tile as tile
from concourse import mybir, bass_utils
import concourse.hw_specs as hw_specs

FP = mybir.dt.float32

# Restrict activation-table selection to one set so the scalar engine never
# reloads tables (everything is built from Exp/Ln/Abs/Relu/Identity/Copy).
_KEEP_TABLES = {"natural_log_exp_and_others"}


def _patched_tables(arch):
    t = hw_specs.get_activation_tables(arch)
    return {k: (v if k in _KEEP_TABLES else set()) for k, v in t.items()}


bacc.get_activation_tables = _patched_tables


# ---------------------------------------------------------------- config
class Cfg:
    def __init__(self, N, M, NG):
        self.NC = 8
        self.N, self.M, self.NG = N, M, NG
        self.FV, self.FE, self.E, self.L = 92, 41, 64, 3
        self.FC0, self.FC1 = 128, 64
        self.ZF = 128                       # z width = 2*E
        self.NB = N // self.NC              # real nodes per core
        self.NBP = -(-(self.NB + 1) // 128) * 128  # padded (>= NB+1: zero row)
        self.NBLK = self.NBP // 128
        self.NT = self.NBP * self.NC
        self.HALF = self.NT // 2
        assert self.HALF - 1 < 32768
        assert self.NBP > self.NB
        self.GS = 16                        # chunks per group (2048 edges)
        self.EPS = 1e-5


# ---------------------------------------------------------- preprocessing
def _wrap_idx16(idx):
    a = idx.reshape(-1, 16).T.astype(np.int16)
    return np.tile(a, (8, 1))


def preprocess(inputs, cfg):
    c = cfg
    src = np.asarray(inputs["src"]).astype(np.int64)
    dst = np.asarray(inputs["dst"]).astype(np.int64)
    ef = np.asarray(inputs["edge_feats"], np.float32)
    nf = np.asarray(inputs["node_feats"], np.float32)
    gid = np.asarray(inputs["graph_ids"]).astype(np.int64)

    pad_row = (src // c.NB) * c.NBP + (src % c.NB)
    owner = dst // c.NB
    dst_loc = dst - owner * c.NB

    cores = []
    for core in range(c.NC):
        em = np.nonzero(owner == core)[0]
        bucket = (pad_row[em] >= c.HALF).astype(np.int64)
        per_bucket = []
        for b in (0, 1):
            eb = em[bucket == b]
            eb = eb[np.argsort(dst_loc[eb], kind="stable")]
            blk = dst_loc[eb] // 128
            segs = []
            for bk in range(c.NBLK):
                run = eb[blk == bk]
                segs.append((run, bk, (-len(run)) % 128))
            per_bucket.append(segs)
        cores.append(per_bucket)

    EP = [0, 0]
    for b in (0, 1):
        for core in range(c.NC):
            tot = sum(len(r) + p for r, _, p in cores[core][b])
            EP[b] = max(EP[b], tot)
        EP[b] = max(-(-EP[b] // 128) * 128, 128)
    EPT = EP[0] + EP[1]
    ZROW = c.NB  # all-zero table row (first pad node), same rel id both halves

    in_maps = []
    for core in range(c.NC):
        srcrel = np.full(EPT, ZROW, np.int64)
        dstrel = np.full(EPT, ZROW, np.int64)
        dstblk = np.full(EPT, -1.0, np.float32)
        blkid = np.zeros(EPT // 128, np.int32)
        eperm = np.full(EPT, -1, np.int64)
        for b in (0, 1):
            boff = b * EP[0]
            pos = 0
            for run, bk, npad in cores[core][b]:
                n = len(run)
                if n:
                    sl = slice(boff + pos, boff + pos + n)
                    srcrel[sl] = pad_row[run] - b * c.HALF
                    dstrel[sl] = dst_loc[run]
                    dstblk[sl] = (dst_loc[run] - bk * 128).astype(np.float32)
                    eperm[sl] = run
                blkid[(boff + pos) // 128: (boff + pos + n + npad) // 128] = bk
                pos += n + npad

        eft = np.zeros((c.FE, EPT), np.float32)
        real = eperm >= 0
        eft[:, real] = ef[eperm[real]].T

        nfT = np.zeros((c.FV, c.NBP), np.float32)
        nfT[:, : c.NB] = nf[core * c.NB: (core + 1) * c.NB].T
        gidc = np.full(c.NBP, -1.0, np.float32)
        gidc[: c.NB] = gid[core * c.NB: (core + 1) * c.NB].astype(np.float32)

        eye = np.eye(129, 128, dtype=np.float32)
        bidx = np.where(dstblk < 0, 128, dstblk.astype(np.int64))
        indt = eye[bidx].reshape(-1, 128, 128)          # [NCH, 128e, 128d]
        m = {
            "srcrel": _wrap_idx16(srcrel.astype(np.int16)),
            "dstrel": _wrap_idx16(dstrel.astype(np.int16)),
            "indt": indt,
            "blkid": blkid.reshape(1, -1),
            "eft": eft,
            "nfT": nfT,
            "gidc": gidc.reshape(-1, 128).T.copy(),
        }
        in_maps.append(m)

    Wm = np.asarray(inputs["Wm"], np.float32)
    Ws = np.asarray(inputs["Ws"], np.float32)
    E = c.E
    shared = {
        "W_emb": np.asarray(inputs["W_emb"], np.float32),
        "g_emb": np.asarray(inputs["g_emb"], np.float32).reshape(1, E),
        "be_emb": np.asarray(inputs["be_emb"], np.float32).reshape(1, E),
        "Wsrc2": np.concatenate([Wm[:, :E, :], Ws[:, :E, :]], axis=2),
        "Wdst2": np.concatenate([Wm[:, E:2 * E, :], Ws[:, E:2 * E, :]], axis=2),
        "Wef2": np.concatenate([Wm[:, 2 * E:, :], Ws[:, 2 * E:, :]], axis=2),
        "gm": np.asarray(inputs["gm"], np.float32),
        "bem": np.asarray(inputs["bem"], np.float32),
        "gs": np.asarray(inputs["gs"], np.float32),
        "bes": np.asarray(inputs["bes"], np.float32),
        "gn": np.asarray(inputs["gn"], np.float32),
        "ben": np.asarray(inputs["ben"], np.float32),
        "Wf0": np.asarray(inputs["Wf0"], np.float32),
        "gf0": np.asarray(inputs["gf0"], np.float32).reshape(-1, 1),
        "bef0": np.asarray(inputs["bef0"], np.float32).reshape(-1, 1),
        "Wf1": np.asarray(inputs["Wf1"], np.float32),
        "gf1": np.asarray(inputs["gf1"], np.float32).reshape(-1, 1),
        "bef1": np.asarray(inputs["bef1"], np.float32).reshape(-1, 1),
        "Wt": np.asarray(inputs["Wt"], np.float32),
        "bt": np.asarray(inputs["bt"], np.float32).reshape(1, 1),
    }
    for m in in_maps:
        m.update(shared)
    return in_maps, EP


# ------------------------------------------------------------- kernel build
def build(cfg, EP):
    c = cfg
    EPT = EP[0] + EP[1]
    NCH = EPT // 128
    DVE = mybir.EngineType.DVE
    AF = mybir.ActivationFunctionType
    OP = mybir.AluOpType

    nc = bacc.Bacc("TRN2", target_bir_lowering=False, debug=False,
                   enable_asserts=False, num_devices=c.NC, num_swdge_queues=4)

    def din(name, shape, dt=FP):
        return nc.dram_tensor(name, shape, dt, kind="ExternalInput")

    t_srcrel = din("srcrel", [128, EPT // 16], mybir.dt.int16)
    t_dstrel = din("dstrel", [128, EPT // 16], mybir.dt.int16)
    t_indt = din("indt", [NCH, 128, 128])
    t_blkid = din("blkid", [1, NCH], mybir.dt.int32)
    t_eft = din("eft", [c.FE, EPT])
    t_nfT = din("nfT", [c.FV, c.NBP])
    t_gidc = din("gidc", [128, c.NBLK])
    t_Wemb = din("W_emb", [c.FV, c.E])
    t_gemb = din("g_emb", [1, c.E])
    t_beemb = din("be_emb", [1, c.E])
    t_Wsrc2 = din("Wsrc2", [c.L, c.E, c.ZF])
    t_Wdst2 = din("Wdst2", [c.L, c.E, c.ZF])
    t_Wef2 = din("Wef2", [c.L, c.FE, c.ZF])
    t_gm = din("gm", [c.L, c.E])
    t_bem = din("bem", [c.L, c.E])
    t_gs = din("gs", [c.L, c.E])
    t_bes = din("bes", [c.L, c.E])
    t_gn = din("gn", [c.L, c.E])
    t_ben = din("ben", [c.L, c.E])
    t_Wf0 = din("Wf0", [c.E, c.FC0])
    t_gf0 = din("gf0", [c.FC0, 1])
    t_bef0 = din("bef0", [c.FC0, 1])
    t_Wf1 = din("Wf1", [c.FC0, c.FC1])
    t_gf1 = din("gf1", [c.FC1, 1])
    t_bef1 = din("bef1", [c.FC1, 1])
    t_Wt = din("Wt", [c.E, 1])
    t_bt = din("bt", [1, 1])
    t_out = nc.dram_tensor("out", [1, c.NG], FP, kind="ExternalOutput")

    RG = [list(range(c.NC))]

    with tile.TileContext(nc) as tc, ExitStack() as es:
        dram = es.enter_context(tc.tile_pool(name="dram", bufs=1, space="DRAM"))
        zbuf = dram.tile([128, NCH, c.ZF], FP)
        adst_dram = dram.tile([c.NBP, c.ZF], FP)
        est_in = [dram.tile([1, 2 * c.ZF], FP, name=f"est_in{i}") for i in range(c.L)]
        est_out = [dram.tile([1, 2 * c.ZF], FP, addr_space="Shared", name=f"est_out{i}")
                   for i in range(c.L)]
        nst_in = [dram.tile([1, 2 * c.E], FP, name=f"nst_in{i}") for i in range(c.L + 1)]
        nst_out = [dram.tile([1, 2 * c.E], FP, addr_space="Shared", name=f"nst_out{i}")
                   for i in range(c.L + 1)]
        agin_l = [dram.tile([c.NBP, c.ZF], FP, name=f"agin{i}") for i in range(c.L)]
        agout_l = [dram.tile([c.NT, c.ZF], FP, addr_space="Shared", name=f"agout{i}")
                   for i in range(c.L)]
        ro_in = dram.tile([c.E + 1, c.NG], FP)
        ro_out = dram.tile([c.E + 1, c.NG], FP, addr_space="Shared")

        konst = es.enter_context(tc.tile_pool(name="konst", bufs=1))
        iotaF = konst.tile([128, 256], FP)
        identF = konst.tile([128, 128], FP)
        ones_row = konst.tile([1, 128], FP)
        ones_col = konst.tile([128, 1], FP)
        epsT = konst.tile([1, 1], FP)
        epsC = konst.tile([128, 1], FP)
        padmask = konst.tile([128, 1], FP)
        with tc.tile_pool(name="ksetup", bufs=1) as ks:
            ii = ks.tile([128, 256], mybir.dt.int32)
            nc.gpsimd.iota(ii[:], pattern=[[1, 256]], base=0, channel_multiplier=0)
            nc.vector.tensor_copy(iotaF[:], ii[:])
            ip = ks.tile([128, 1], mybir.dt.int32)
            nc.gpsimd.iota(ip[:], pattern=[[1, 1]], base=0, channel_multiplier=1)
            ipf = ks.tile([128, 1], FP)
            nc.vector.tensor_copy(ipf[:], ip[:])
            nc.vector.tensor_scalar(identF[:], iotaF[:, :128], ipf[:], None, OP.is_equal)
            nc.vector.tensor_scalar(padmask[:], ipf[:], float(c.NB % 128), None, OP.is_lt)
        nc.vector.memset(ones_row[:], 1.0)
        nc.vector.memset(ones_col[:], 1.0)
        nc.vector.memset(epsT[:], c.EPS)
        nc.vector.memset(epsC[:], c.EPS)

        state = es.enter_context(tc.tile_pool(name="state", bufs=1))
        v_sb = state.tile([128, c.NBLK, c.E], FP)
        agg_sb = state.tile([128, c.NBLK, c.E], FP)
        blkid_sb = state.tile([1, NCH], mybir.dt.int32)
        gid_sb = state.tile([128, c.NBLK], FP)
        nc.sync.dma_start(blkid_sb[:], t_blkid[:])
        nc.sync.dma_start(gid_sb[:], t_gidc[:])

        wts = es.enter_context(tc.tile_pool(name="wts", bufs=1))
        Wsrc2_sb = wts.tile([c.E, c.L * c.ZF], FP)
        Wdst2_sb = wts.tile([c.E, c.L * c.ZF], FP)
        Wef2_sb = wts.tile([c.FE, c.L * c.ZF], FP)
        for l in range(c.L):
            nc.sync.dma_start(Wsrc2_sb[:, l * c.ZF:(l + 1) * c.ZF], t_Wsrc2[l])
            nc.sync.dma_start(Wdst2_sb[:, l * c.ZF:(l + 1) * c.ZF], t_Wdst2[l])
            nc.sync.dma_start(Wef2_sb[:, l * c.ZF:(l + 1) * c.ZF], t_Wef2[l])

        # sigmoid(x) -> out, via one act table: sig = exp(-softplus(-x))
        def sigmoid_ops(pool, out, x, shape, nm):
            t1 = pool.tile(shape, FP, name=f"sgA{nm}", tag=f"sgA{nm}")
            nc.scalar.activation(t1[:], x, AF.Abs)
            nc.scalar.activation(t1[:], t1[:], AF.Exp, scale=-1.0)
            nc.any.tensor_scalar_add(t1[:], t1[:], 1.0)
            nc.scalar.activation(t1[:], t1[:], AF.Ln)
            t2 = pool.tile(shape, FP, name=f"sgB{nm}", tag=f"sgB{nm}")
            nc.vector.tensor_scalar(t2[:], x, 0.0, -1.0, OP.min, OP.mult)
            nc.any.tensor_add(t1[:], t1[:], t2[:])
            nc.scalar.activation(out, t1[:], AF.Exp, scale=-1.0)

        # softplus(x) -> out = ln(1+exp(-|x|)) + relu(x)
        def softplus_ops(pool, out, x, shape, nm):
            t1 = pool.tile(shape, FP, name=f"spA{nm}", tag=f"spA{nm}")
            nc.scalar.activation(t1[:], x, AF.Abs)
            nc.scalar.activation(t1[:], t1[:], AF.Exp, scale=-1.0)
            nc.any.tensor_scalar_add(t1[:], t1[:], 1.0)
            nc.scalar.activation(t1[:], t1[:], AF.Ln)
            t2 = pool.tile(shape, FP, name=f"spB{nm}", tag=f"spB{nm}")
            nc.scalar.activation(t2[:], x, AF.Relu)
            nc.any.tensor_add(out, t1[:], t2[:])

        def bn_fold(pool, sums, F, count, g_ap, be_ap):
            st = pool.tile([1, 2 * F], FP, name=f"bnf{nc.next_id()}")
            mean = pool.tile([1, F], FP, name=f"bnm{nc.next_id()}")
            var = pool.tile([1, F], FP, name=f"bnv{nc.next_id()}")
            nc.scalar.mul(mean[:], sums[:, 0:F], 1.0 / count)
            nc.scalar.mul(var[:], sums[:, F:2 * F], 1.0 / count)
            m2 = pool.tile([1, F], FP, name=f"bn2{nc.next_id()}")
            nc.vector.tensor_mul(m2[:], mean[:], mean[:])
            nc.vector.tensor_sub(var[:], var[:], m2[:])
            nc.scalar.activation(var[:], var[:], AF.Ln, bias=epsT[0:1, 0:1])
            nc.scalar.activation(var[:], var[:], AF.Exp, scale=-0.5)
            nc.vector.tensor_mul(st[:, 0:F], g_ap, var[:])
            nc.vector.tensor_mul(mean[:], mean[:], st[:, 0:F])
            nc.vector.tensor_sub(st[:, F:2 * F], be_ap, mean[:])
            return st

        def bcast_row(pool, psum_pool, row_ap, W, name):
            ps = psum_pool.tile([128, W], FP, name=f"ps{name}")
            nc.tensor.matmul(ps[:], ones_row[:, :], row_ap, start=True, stop=True)
            sb = pool.tile([128, W], FP, name=name)
            nc.scalar.copy(sb[:], ps[:])
            return sb

        def zero_vpad():
            # zero pad-node rows of the last block (per-partition mask multiply)
            cb = c.NB // 128
            nc.vector.tensor_scalar(v_sb[:, cb, :], v_sb[:, cb, :],
                                    padmask[:], None, OP.mult)

        # ---------------------------------------------------- embedding
        with tc.tile_pool(name="emb", bufs=1) as emb, \
             tc.tile_pool(name="embw", bufs=2) as embw, \
             tc.tile_pool(name="embp", bufs=2, space="PSUM") as embp, \
             tc.tile_pool(name="embs", bufs=1, space="PSUM") as embs:
            nfT_sb = emb.tile([c.FV, c.NBP], FP)
            nc.sync.dma_start(nfT_sb[:], t_nfT[:])
            Wemb_sb = emb.tile([c.FV, c.E], FP)
            nc.sync.dma_start(Wemb_sb[:], t_Wemb[:])
            z0 = emb.tile([128, c.NBLK, c.E], FP)
            ssum = embs.tile([1, c.E], FP)
            ssq = embs.tile([1, c.E], FP)
            for ch in range(c.NBLK):
                ps = embp.tile([128, c.E], FP, name="embz")
                nc.tensor.matmul(ps[:], nfT_sb[:, ch * 128:(ch + 1) * 128],
                                 Wemb_sb[:], start=True, stop=True)
                nc.scalar.copy(z0[:, ch, :], ps[:])
                sq = embw.tile([128, c.E], FP, name="embsq")
                nc.vector.tensor_mul(sq[:], z0[:, ch, :], z0[:, ch, :])
                nc.tensor.matmul(ssum[:], ones_col[:, :], z0[:, ch, :],
                                 start=(ch == 0), stop=(ch == c.NBLK - 1))
                nc.tensor.matmul(ssq[:], ones_col[:, :], sq[:],
                                 start=(ch == 0), stop=(ch == c.NBLK - 1))
            stat = emb.tile([1, 2 * c.E], FP)
            nc.vector.tensor_copy(stat[:, 0:c.E], ssum[:])
            nc.vector.tensor_copy(stat[:, c.E:], ssq[:])
            nc.sync.dma_start(nst_in[c.L][:], stat[:])
            nc.gpsimd.collective_compute(
                "AllReduce", OP.add, replica_groups=RG,
                ins=[nst_in[c.L].opt()], outs=[nst_out[c.L].opt()])
            rstat = emb.tile([1, 2 * c.E], FP)
            nc.sync.dma_start(rstat[:], nst_out[c.L][:])
            gemb_sb = emb.tile([1, c.E], FP)
            beemb_sb = emb.tile([1, c.E], FP)
            nc.sync.dma_start(gemb_sb[:], t_gemb[:])
            nc.sync.dma_start(beemb_sb[:], t_beemb[:])
            st = bn_fold(emb, rstat, c.E, c.N, gemb_sb[:], beemb_sb[:])
            stb = bcast_row(emb, embp, st[:], 2 * c.E, "embst")
            for ch in range(c.NBLK):
                u = embw.tile([128, c.E], FP, name="embu")
                nc.vector.tensor_mul(u[:], z0[:, ch, :], stb[:, 0:c.E])
                nc.vector.tensor_add(u[:], u[:], stb[:, c.E:])
                sg = embw.tile([128, c.E], FP, name="embsg")
                sigmoid_ops(embw, sg[:], u[:], [128, c.E], "emb")
                nc.vector.tensor_mul(v_sb[:, ch, :], u[:], sg[:])
            zero_vpad()

        # ---------------------------------------------------- conv layers
        gq = 0
        for l in range(c.L):
            # ---- phase A: projection tables
            with tc.tile_pool(name="phA", bufs=2) as pa, \
                 tc.tile_pool(name="phAp", bufs=2, space="PSUM") as pap, \
                 tc.tile_pool(name="phAo", bufs=2, space="PSUM") as pao:
                agin_sb = pa.tile([128, c.NBLK, c.ZF], FP, bufs=1)
                adst_sb = pa.tile([128, c.NBLK, c.ZF], FP, bufs=1)
                for ch in range(c.NBLK):
                    vt_ps = pap.tile([c.E, 128], FP, name="vtps")
                    nc.tensor.transpose(vt_ps[:], v_sb[:, ch, :], identF[:])
                    vt = pa.tile([c.E, 128], FP, name="vt")
                    nc.scalar.copy(vt[:], vt_ps[:])
                    a1 = pao.tile([128, c.ZF], FP, name="a1")
                    nc.tensor.matmul(a1[:], vt[:], Wsrc2_sb[:, l * c.ZF:(l + 1) * c.ZF],
                                     start=True, stop=True)
                    nc.scalar.copy(agin_sb[:, ch, :], a1[:])
                    a2 = pao.tile([128, c.ZF], FP, name="a2")
                    nc.tensor.matmul(a2[:], vt[:], Wdst2_sb[:, l * c.ZF:(l + 1) * c.ZF],
                                     start=True, stop=True)
                    nc.vector.tensor_copy(adst_sb[:, ch, :], a2[:])
                nc.sync.dma_start(
                    agin_l[l][:].rearrange("(b p) f -> p b f", p=128), agin_sb[:])
                nc.sync.dma_start(
                    adst_dram[:].rearrange("(b p) f -> p b f", p=128), adst_sb[:])
            nc.gpsimd.collective_compute(
                "AllGather", OP.bypass, replica_groups=RG,
                ins=[agin_l[l].opt()], outs=[agout_l[l].opt()])

            # ---- pass 1: z + stats
            with tc.tile_pool(name="p1idx", bufs=2) as pidx, \
                 tc.tile_pool(name="p1g", bufs=3) as pg, \
                 tc.tile_pool(name="p1z", bufs=2) as pz, \
                 tc.tile_pool(name="p1acc", bufs=1) as pacc, \
                 tc.tile_pool(name="p1zp", bufs=4, space="PSUM") as pzp:
                acc_z = pacc.tile([128, c.GS, c.ZF], FP)
                acc_q = pacc.tile([128, c.GS, c.ZF], FP)
                nc.vector.memset(acc_z[:], 0.0)
                nc.vector.memset(acc_q[:], 0.0)
                for b in (0, 1):
                    nchb = EP[b] // 128
                    base_ch = (0 if b == 0 else EP[0] // 128)
                    for g0 in range(0, nchb, c.GS):
                        gs = min(c.GS, nchb - g0)
                        ni = gs * 128
                        coff = base_ch + g0
                        idxs_t = pidx.tile([128, c.GS * 8], mybir.dt.int16, name="idxs")
                        nc.sync.dma_start(idxs_t[:, :gs * 8],
                                          t_srcrel[:, coff * 8:coff * 8 + gs * 8])
                        idxd_t = pidx.tile([128, c.GS * 8], mybir.dt.int16, name="idxd")
                        nc.sync.dma_start(idxd_t[:, :gs * 8],
                                          t_dstrel[:, coff * 8:coff * 8 + gs * 8])
                        gsr_t = pg.tile([128, c.GS, c.ZF], FP, name="gsrc")
                        nc.gpsimd.dma_gather(
                            gsr_t[:, :gs, :],
                            agout_l[l][b * c.HALF:(b + 1) * c.HALF, :],
                            idxs_t[:, :gs * 8], num_idxs=ni, num_idxs_reg=ni,
                            elem_size=c.ZF, queue_num=gq % 4, single_packet=False)
                        gq += 1
                        gds_t = pg.tile([128, c.GS, c.ZF], FP, name="gdst")
                        nc.gpsimd.dma_gather(
                            gds_t[:, :gs, :],
                            adst_dram[:],
                            idxd_t[:, :gs * 8], num_idxs=ni, num_idxs_reg=ni,
                            elem_size=c.ZF, queue_num=gq % 4, single_packet=False)
                        gq += 1
                        ef_t = pg.tile([c.FE, c.GS * 128], FP, name="eft")
                        nc.sync.dma_start(ef_t[:, :ni],
                                          t_eft[:, coff * 128:coff * 128 + ni])
                        pef_t = pz.tile([128, c.GS, c.ZF], FP, name="peft")
                        for j in range(gs):
                            ps = pzp.tile([128, c.ZF], FP, name="psz")
                            nc.tensor.matmul(ps[:], ef_t[:, j * 128:(j + 1) * 128],
                                             Wef2_sb[:, l * c.ZF:(l + 1) * c.ZF],
                                             start=True, stop=True)
                            nc.scalar.copy(pef_t[:, j, :], ps[:])
                        z_t = pz.tile([128, c.GS, c.ZF], FP, name="zt")
                        nc.any.tensor_add(z_t[:, :gs, :], gsr_t[:, :gs, :], pef_t[:, :gs, :])
                        nc.any.tensor_add(z_t[:, :gs, :], z_t[:, :gs, :], gds_t[:, :gs, :])
                        nc.sync.dma_start(zbuf[:, coff:coff + gs, :], z_t[:, :gs, :])
                        sq_t = pz.tile([128, c.GS, c.ZF], FP, name="sqt")
                        nc.any.tensor_mul(sq_t[:, :gs, :], z_t[:, :gs, :], z_t[:, :gs, :])
                        nc.any.tensor_add(acc_z[:, :gs, :], acc_z[:, :gs, :], z_t[:, :gs, :])
                        nc.any.tensor_add(acc_q[:, :gs, :], acc_q[:, :gs, :], sq_t[:, :gs, :])
                with tc.tile_pool(name="p1st", bufs=1) as pst, \
                     tc.tile_pool(name="p1sp", bufs=1, space="PSUM") as psp:
                    red_z = pst.tile([128, c.ZF], FP)
                    red_q = pst.tile([128, c.ZF], FP)
                    nc.vector.tensor_reduce(
                        red_z[:], acc_z[:].rearrange("p g f -> p f g"),
                        mybir.AxisListType.X, OP.add)
                    nc.vector.tensor_reduce(
                        red_q[:], acc_q[:].rearrange("p g f -> p f g"),
                        mybir.AxisListType.X, OP.add)
                    pss = psp.tile([1, c.ZF], FP, name="pss")
                    psq = psp.tile([1, c.ZF], FP, name="psq")
                    nc.tensor.matmul(pss[:], ones_col[:, :], red_z[:], start=True, stop=True)
                    nc.tensor.matmul(psq[:], ones_col[:, :], red_q[:], start=True, stop=True)
                    stat = pst.tile([1, 2 * c.ZF], FP)
                    nc.vector.tensor_copy(stat[:, :c.ZF], pss[:])
                    nc.vector.tensor_copy(stat[:, c.ZF:], psq[:])
                    nc.sync.dma_start(est_in[l][:], stat[:])

            nc.gpsimd.collective_compute(
                "AllReduce", OP.add, replica_groups=RG,
                ins=[est_in[l].opt()], outs=[est_out[l].opt()])

            # ---- pass 2: activations + scatter
            with tc.tile_pool(name="p2", bufs=1) as p2, \
                 tc.tile_pool(name="p2z", bufs=2) as p2z, \
                 tc.tile_pool(name="p2w", bufs=3) as p2w, \
                 tc.tile_pool(name="p2ap", bufs=4, space="PSUM") as p2ap, \
                 tc.tile_pool(name="p2bp", bufs=1, space="PSUM") as p2bp:
                rstat = p2.tile([1, 2 * c.ZF], FP)
                nc.sync.dma_start(rstat[:], est_out[l][:])
                gms = p2.tile([1, 2 * c.E], FP)
                nc.sync.dma_start(gms[:, :c.E], t_gm[l:l + 1, :])
                nc.sync.dma_start(gms[:, c.E:], t_gs[l:l + 1, :])
                bms = p2.tile([1, 2 * c.E], FP)
                nc.sync.dma_start(bms[:, :c.E], t_bem[l:l + 1, :])
                nc.sync.dma_start(bms[:, c.E:], t_bes[l:l + 1, :])
                st = bn_fold(p2, rstat, c.ZF, c.M, gms[:], bms[:])
                stb = bcast_row(p2, p2bp, st[:], 2 * c.ZF, "edgest")
                s_g = p2.tile([128, c.GS, c.ZF], FP)
                t_g = p2.tile([128, c.GS, c.ZF], FP)
                for j in range(c.GS):
                    nc.vector.tensor_copy(s_g[:, j, :], stb[:, 0:c.ZF])
                    nc.vector.tensor_copy(t_g[:, j, :], stb[:, c.ZF:])
                nc.vector.memset(agg_sb[:], 0.0)
                for b in (0, 1):
                    nchb = EP[b] // 128
                    base_ch = (0 if b == 0 else EP[0] // 128)
                    for g0 in range(0, nchb, c.GS):
                        gs = min(c.GS, nchb - g0)
                        coff = base_ch + g0
                        z_t = p2z.tile([128, c.GS, c.ZF], FP, name="z2t")
                        nc.sync.dma_start(z_t[:, :gs, :], zbuf[:, coff:coff + gs, :])
                        ind_t = p2z.tile([128, c.GS, 128], FP, name="indt")
                        nc.sync.dma_start(
                            ind_t[:, :gs, :],
                            t_indt[coff:coff + gs].rearrange("c p d -> p c d"))
                        u = p2z.tile([128, c.GS, c.ZF], FP, name="u")
                        nc.any.tensor_mul(u[:, :gs, :], z_t[:, :gs, :], s_g[:, :gs, :])
                        nc.any.tensor_add(u[:, :gs, :], u[:, :gs, :], t_g[:, :gs, :])
                        um = u[:, :gs, 0:c.E]
                        us = u[:, :gs, c.E:]
                        # core = ln(1 + exp(-|u|)) on both halves at once
                        core = p2z.tile([128, c.GS, c.ZF], FP, name="core")
                        nc.scalar.activation(core[:, :gs, :], u[:, :gs, :], AF.Abs)
                        nc.scalar.activation(core[:, :gs, :], core[:, :gs, :], AF.Exp, scale=-1.0)
                        nc.any.tensor_scalar_add(core[:, :gs, :], core[:, :gs, :], 1.0)
                        nc.scalar.activation(core[:, :gs, :], core[:, :gs, :], AF.Ln)
                        # softplus(us) = core_s + relu(us)
                        sp = p2z.tile([128, c.GS, c.E], FP, name="sp")
                        nc.scalar.activation(sp[:, :gs, :], us, AF.Relu)
                        nc.any.tensor_add(sp[:, :gs, :], sp[:, :gs, :], core[:, :gs, c.E:])
                        # sigmoid(um) = exp(-(core_m + relu(-um)))
                        sg = p2z.tile([128, c.GS, c.E], FP, name="sg")
                        nc.vector.tensor_scalar(sg[:, :gs, :], um, 0.0, -1.0, OP.min, OP.mult)
                        nc.any.tensor_add(sg[:, :gs, :], sg[:, :gs, :], core[:, :gs, 0:c.E])
                        nc.scalar.activation(sg[:, :gs, :], sg[:, :gs, :], AF.Exp, scale=-1.0)
                        h = p2z.tile([128, c.GS, c.E], FP, name="h")
                        nc.any.tensor_mul(h[:, :gs, :], sg[:, :gs, :], sp[:, :gs, :])
                        for j in range(gs):
                            ch = coff + j
                            ag = p2ap.tile([128, c.E], FP, name="psagg")
                            nc.tensor.matmul(ag[:], ind_t[:, j, :], h[:, j, :],
                                             start=True, stop=True)
                            r = nc.alloc_registers(engines=[DVE])
                            nc.regs_load(r, blkid_sb[0:1, ch:ch + 1])
                            bv = nc.snap(r, donate=True, min_val=0, max_val=c.NBLK - 1)
                            sl = agg_sb[:].rearrange("p b f -> p (b f)")[:, bass.ts(bv, c.E)]
                            nc.vector.tensor_tensor(sl, sl, ag[:], OP.add)

            # ---- node BN + update
            with tc.tile_pool(name="nod", bufs=1) as nod, \
                 tc.tile_pool(name="nodw", bufs=2) as nodw, \
                 tc.tile_pool(name="nodp", bufs=2, space="PSUM") as nodp, \
                 tc.tile_pool(name="nods", bufs=1, space="PSUM") as nods:
                nsum = nods.tile([1, c.E], FP)
                nssq = nods.tile([1, c.E], FP)
                for ch in range(c.NBLK):
                    sq = nodw.tile([128, c.E], FP, name="nsq")
                    nc.vector.tensor_mul(sq[:], agg_sb[:, ch, :], agg_sb[:, ch, :])
                    nc.tensor.matmul(nsum[:], ones_col[:, :], agg_sb[:, ch, :],
                                     start=(ch == 0), stop=(ch == c.NBLK - 1))
                    nc.tensor.matmul(nssq[:], ones_col[:, :], sq[:],
                                     start=(ch == 0), stop=(ch == c.NBLK - 1))
                stat = nod.tile([1, 2 * c.E], FP)
                nc.vector.tensor_copy(stat[:, :c.E], nsum[:])
                nc.vector.tensor_copy(stat[:, c.E:], nssq[:])
                nc.sync.dma_start(nst_in[l][:], stat[:])
                nc.gpsimd.collective_compute(
                    "AllReduce", OP.add, replica_groups=RG,
                    ins=[nst_in[l].opt()], outs=[nst_out[l].opt()])
                rstat = nod.tile([1, 2 * c.E], FP)
                nc.sync.dma_start(rstat[:], nst_out[l][:])
                gn_sb = nod.tile([1, c.E], FP)
                ben_sb = nod.tile([1, c.E], FP)
                nc.sync.dma_start(gn_sb[:], t_gn[l:l + 1, :])
                nc.sync.dma_start(ben_sb[:], t_ben[l:l + 1, :])
                st = bn_fold(nod, rstat, c.E, c.N, gn_sb[:], ben_sb[:])
                stb = bcast_row(nod, nodp, st[:], 2 * c.E, "nodst")
                for ch in range(c.NBLK):
                    u = nodw.tile([128, c.E], FP, name="nu")
                    nc.vector.tensor_mul(u[:], agg_sb[:, ch, :], stb[:, 0:c.E])
                    nc.vector.tensor_add(u[:], u[:], stb[:, c.E:])
                    nc.vector.tensor_add(u[:], u[:], v_sb[:, ch, :])
                    softplus_ops(nodw, v_sb[:, ch, :], u[:], [128, c.E], "nod")
                zero_vpad()

        # ---------------------------------------------------- readout
        with tc.tile_pool(name="ro", bufs=1) as ro, \
             tc.tile_pool(name="row", bufs=2) as row, \
             tc.tile_pool(name="rop", bufs=1, space="PSUM") as rop, \
             tc.tile_pool(name="ros", bufs=1, space="PSUM") as ros:
            psums = ros.tile([c.E, c.NG], FP)
            pcnt = ros.tile([1, c.NG], FP)
            for ch in range(c.NBLK):
                gind = row.tile([128, c.NG], FP, name="gind")
                nc.vector.tensor_scalar(gind[:], iotaF[:, :c.NG],
                                        gid_sb[:, ch:ch + 1], None, OP.is_equal)
                nc.tensor.matmul(psums[:], v_sb[:, ch, :], gind[:],
                                 start=(ch == 0), stop=(ch == c.NBLK - 1))
                nc.tensor.matmul(pcnt[:], ones_col[:, :], gind[:],
                                 start=(ch == 0), stop=(ch == c.NBLK - 1))
            acc = ro.tile([c.E + 1, c.NG], FP)
            nc.scalar.copy(acc[0:c.E, :], psums[:])
            nc.scalar.copy(acc[c.E:c.E + 1, :], pcnt[:])
            nc.sync.dma_start(ro_in[:], acc[:])
            nc.gpsimd.collective_compute(
                "AllReduce", OP.add, replica_groups=RG,
                ins=[ro_in.opt()], outs=[ro_out.opt()])
            racc = ro.tile([c.E + 1, c.NG], FP)
            nc.sync.dma_start(racc[:], ro_out[:])
            cnt = ro.tile([1, c.NG], FP)
            nc.vector.tensor_scalar_max(cnt[:], racc[c.E:c.E + 1, :], 1.0)
            nc.vector.reciprocal(cnt[:], cnt[:])
            rcb_ps = rop.tile([c.E, c.NG], FP, name="rcb", tag="rosc")
            nc.tensor.matmul(rcb_ps[:], ones_row[:, 0:c.E], cnt[:], start=True, stop=True)
            vs = ro.tile([c.E, c.NG], FP)
            nc.vector.tensor_tensor(vs[:], racc[0:c.E, :], rcb_ps[:], OP.mult)

            def fc_bn_silu(pool, psum_pool, x_sb, W_ap, K, Fo, g_t, be_t, nm):
                ps = psum_pool.tile([Fo, c.NG], FP, name=f"fc{nm}", tag="rosc")
                W_sb = pool.tile([K, Fo], FP, name=f"W{nm}")
                nc.sync.dma_start(W_sb[:], W_ap)
                nc.tensor.matmul(ps[:], W_sb[:], x_sb[:], start=True, stop=True)
                g_sb = pool.tile([Fo, 1], FP, name=f"g{nm}")
                be_sb = pool.tile([Fo, 1], FP, name=f"be{nm}")
                nc.sync.dma_start(g_sb[:], g_t[:])
                nc.sync.dma_start(be_sb[:], be_t[:])
                x_sbc = pool.tile([Fo, c.NG], FP, name=f"x{nm}")
                nc.scalar.copy(x_sbc[:], ps[:])
                sums = pool.tile([Fo, 1], FP, name=f"su{nm}")
                nc.vector.tensor_reduce(sums[:], x_sbc[:], mybir.AxisListType.X, OP.add)
                sq = pool.tile([Fo, c.NG], FP, name=f"sq{nm}")
                nc.vector.tensor_mul(sq[:], x_sbc[:], x_sbc[:])
                ssq = pool.tile([Fo, 1], FP, name=f"sl{nm}")
                nc.vector.tensor_reduce(ssq[:], sq[:], mybir.AxisListType.X, OP.add)
                mean = pool.tile([Fo, 1], FP, name=f"mn{nm}")
                nc.scalar.mul(mean[:], sums[:], 1.0 / c.NG)
                var = pool.tile([Fo, 1], FP, name=f"vr{nm}")
                nc.scalar.mul(var[:], ssq[:], 1.0 / c.NG)
                m2 = pool.tile([Fo, 1], FP, name=f"m2{nm}")
                nc.vector.tensor_mul(m2[:], mean[:], mean[:])
                nc.vector.tensor_sub(var[:], var[:], m2[:])
                nc.scalar.activation(var[:], var[:], AF.Ln, bias=epsC[0:Fo, :])
                nc.scalar.activation(var[:], var[:], AF.Exp, scale=-0.5)
                s_col = pool.tile([Fo, 1], FP, name=f"sc{nm}")
                nc.vector.tensor_mul(s_col[:], g_sb[:], var[:])
                t_col = pool.tile([Fo, 1], FP, name=f"tc{nm}")
                nc.vector.tensor_mul(t_col[:], mean[:], s_col[:])
                nc.vector.tensor_sub(t_col[:], be_sb[:], t_col[:])
                u = pool.tile([Fo, c.NG], FP, name=f"u{nm}")
                nc.scalar.activation(u[:], x_sbc[:], AF.Identity,
                                     bias=t_col[:], scale=s_col[:])
                sg2 = pool.tile([Fo, c.NG], FP, name=f"sg{nm}")
                sigmoid_ops(pool, sg2[:], u[:], [Fo, c.NG], f"fc{nm}")
                out = pool.tile([Fo, c.NG], FP, name=f"o{nm}")
                nc.vector.tensor_mul(out[:], u[:], sg2[:])
                return out

            z1 = fc_bn_silu(ro, rop, vs, t_Wf0[:], c.E, c.FC0, t_gf0, t_bef0, "0")
            z2 = fc_bn_silu(ro, rop, z1, t_Wf1[:], c.FC0, c.FC1, t_gf1, t_bef1, "1")
            Wt_sb = ro.tile([c.E, 1], FP)
            nc.sync.dma_start(Wt_sb[:], t_Wt[:])
            hd = rop.tile([1, c.NG], FP, name="hd", tag="rosc")
            nc.tensor.matmul(hd[:], Wt_sb[:], z2[:], start=True, stop=True)
            bt_sb = ro.tile([1, 1], FP)
            nc.sync.dma_start(bt_sb[:], t_bt[:])
            res = ro.tile([1, c.NG], FP)
            nc.vector.tensor_scalar(res[:], hd[:], bt_sb[0:1, 0:1], None, OP.add)
            nc.sync.dma_start(t_out[:], res[:])

    nc.compile()
    return nc


# ------------------------------------------------------------------ driver
_CACHE = {}


def kernel(**inputs):
    cfg = Cfg(int(inputs["node_feats"].shape[0]),
              int(inputs["src"].shape[0]), 256)
    in_maps, EP = preprocess(inputs, cfg)
    key = (cfg.N, cfg.M, tuple(EP))
    if key not in _CACHE:
        _CACHE[key] = build(cfg, EP)
    nc = _CACHE[key]
    res = bass_utils.run_bass_kernel_spmd(
        nc, in_maps, core_ids=list(range(cfg.NC)), trace=False)
    out = np.asarray(res.results[0]["out"], np.float32)
    return out.reshape(cfg.NG, 1)



# revision 12
# speedup vs baseline: 1.7032x; 1.7032x over previous
"""CGCNN (gnn_message_passing) Trainium2 kernel — 8-core SPMD, bf16 edge path.

Strategy (v2):
  - Nodes partitioned contiguously across 8 cores (6250/core, padded to 6272);
    edges assigned to the dst-owner core, sorted by dst, grouped into 128-edge
    chunks that never cross a 128-node dst block.  Per-(bucket, block) segment
    lengths are padded to the max across cores so the chunk->block map is
    STATIC and identical on all cores (enables per-block PSUM accumulation of
    the scatter instead of register-indexed dynamic adds).
  - Per conv layer each core computes [A_src|A_dst] = v @ [Wsrc|Wdst] per
    128-node block (one bf16 matmul per block).  A_src is AllGathered (bf16);
    A_dst stays resident in SBUF.
  - Pass 1 (per 32-chunk group): dma_gather of A_src rows by edge src (bf16,
    256B rows, SWDGE 4 queues); z = gather + PSUM(ef-proj matmul + dst-expand
    matmul via host-shipped bf16 indicator M_de against SBUF A_dst); z spilled
    to DRAM in bf16.  Edge-BN stats: sum(z) comes from a degree-weighted
    reduction of the local tables (host ships degree columns + the constant
    ef-projection sum), so only sum(z^2) is accumulated per edge.
  - Pass 2: reload z, folded BN affine + sigmoid/softplus built from
    Exp/Ln/Abs/Relu (one activation table), h = sig*softplus, scatter via
    per-chunk indicator matmuls (M_ed, host-shipped bf16) accumulated in PSUM
    per dst block (static!), flushed once per block run into the fp32 agg.
  - Node BN: local sums + tiny AllReduce.  Readout (per-graph mean + 2 MLPs +
    head) computed redundantly per core via graph-indicator matmuls + one
    small AllReduce.  Linear biases feeding BN cancel and are ignored.
"""

import sys
import os
from contextlib import ExitStack

sys.path.insert(0, "/opt/trn_rl_repo")

import numpy as np
import ml_dtypes

BF_NP = ml_dtypes.bfloat16

import concourse.bass as bass
import concourse.bacc as bacc
import concourse.tile as tile
from concourse import mybir, bass_utils
import concourse.hw_specs as hw_specs

FP = mybir.dt.float32
BF = mybir.dt.bfloat16

# Restrict activation-table selection to one set so the scalar engine never
# reloads tables (everything is built from Exp/Ln/Abs/Relu/Square/Identity).
_KEEP_TABLES = {"natural_log_exp_and_others"}


def _patched_tables(arch):
    t = hw_specs.get_activation_tables(arch)
    return {k: (v if k in _KEEP_TABLES else set()) for k, v in t.items()}


bacc.get_activation_tables = _patched_tables


# ---------------------------------------------------------------- config
class Cfg:
    def __init__(self, N, M, NG):
        self.NC = 8
        self.N, self.M, self.NG = N, M, NG
        self.FV, self.FE, self.E, self.L = 92, 41, 64, 3
        self.FC0, self.FC1 = 128, 64
        self.ZF = 128                       # z width = 2*E
        self.NB = N // self.NC              # real nodes per core
        self.NBP = -(-(self.NB + 1) // 128) * 128  # padded (>= NB+1: zero row)
        self.NBLK = self.NBP // 128
        self.NT = self.NBP * self.NC
        self.HALF = self.NT // 2
        assert self.HALF - 1 < 32768
        assert self.NBP > self.NB
        self.GS = 32                        # chunks per group (4096 edges)
        self.EPS = 1e-5
        self.blkid = None                   # set by preprocess (static layout)


# ---------------------------------------------------------- preprocessing
def _wrap_idx16(idx):
    a = idx.reshape(-1, 16).T.astype(np.int16)
    return np.tile(a, (8, 1))


def preprocess(inputs, cfg):
    c = cfg
    src = np.asarray(inputs["src"]).astype(np.int64)
    dst = np.asarray(inputs["dst"]).astype(np.int64)
    ef = np.asarray(inputs["edge_feats"], np.float32)
    nf = np.asarray(inputs["node_feats"], np.float32)
    gid = np.asarray(inputs["graph_ids"]).astype(np.int64)

    pad_row = (src // c.NB) * c.NBP + (src % c.NB)
    owner = dst // c.NB
    dst_loc = dst - owner * c.NB

    # per-core, per-bucket, per-block edge runs (sorted by dst within block)
    cores = []
    for core in range(c.NC):
        em = np.nonzero(owner == core)[0]
        bucket = (pad_row[em] >= c.HALF).astype(np.int64)
        per_bucket = []
        for b in (0, 1):
            eb = em[bucket == b]
            eb = eb[np.argsort(dst_loc[eb], kind="stable")]
            blk = dst_loc[eb] // 128
            segs = []
            for bk in range(c.NBLK):
                segs.append(eb[blk == bk])
            per_bucket.append(segs)
        cores.append(per_bucket)

    # pad each (bucket, block) segment to the max across cores (STATIC layout)
    PL = np.zeros((2, c.NBLK), np.int64)
    for b in (0, 1):
        for bk in range(c.NBLK):
            mx = max(len(cores[core][b][bk]) for core in range(c.NC))
            PL[b, bk] = -(-mx // 128) * 128 if mx > 0 else 0
    EP = [int(PL[0].sum()), int(PL[1].sum())]
    EPT = EP[0] + EP[1]
    NCH = EPT // 128
    ZROW = c.NB  # all-zero table row (first pad node), same rel id both halves

    # static chunk -> block map (identical across cores)
    blkid = []
    for b in (0, 1):
        for bk in range(c.NBLK):
            blkid.extend([bk] * (PL[b, bk] // 128))
    assert len(blkid) == NCH
    cfg.blkid = tuple(blkid)

    outdeg = np.bincount(src, minlength=c.N).astype(np.float32)
    indeg = np.bincount(dst, minlength=c.N).astype(np.float32)

    Wm = np.asarray(inputs["Wm"], np.float32)
    Ws = np.asarray(inputs["Ws"], np.float32)
    E = c.E
    Wsrc2 = np.concatenate([Wm[:, :E, :], Ws[:, :E, :]], axis=2)       # [L,64,128]
    Wdst2 = np.concatenate([Wm[:, E:2 * E, :], Ws[:, E:2 * E, :]], axis=2)
    Wef2 = np.concatenate([Wm[:, 2 * E:, :], Ws[:, 2 * E:, :]], axis=2)  # [L,41,128]
    Wsd2 = np.concatenate([Wsrc2, Wdst2], axis=2)                       # [L,64,256]

    iota128 = np.arange(128, dtype=np.int64)

    in_maps = []
    for core in range(c.NC):
        srcrel = np.full(EPT, ZROW, np.int64)
        dstoff = np.full(EPT, -1, np.int64)
        eperm = np.full(EPT, -1, np.int64)
        pos = 0
        for b in (0, 1):
            for bk in range(c.NBLK):
                run = cores[core][b][bk]
                n = len(run)
                if n:
                    sl = slice(pos, pos + n)
                    srcrel[sl] = pad_row[run] - b * c.HALF
                    dstoff[sl] = dst_loc[run] - bk * 128
                    eperm[sl] = run
                pos += PL[b, bk]
        assert pos == EPT

        eft = np.zeros((c.FE, EPT), np.float32)
        real = eperm >= 0
        eft[:, real] = ef[eperm[real]].T

        D = dstoff.reshape(NCH, 128)                       # [c, e]
        ind_ced = D[:, :, None] == iota128[None, None, :]  # [c, e, d]
        indt = np.ascontiguousarray(
            ind_ced.transpose(1, 0, 2)).reshape(128, NCH * 128)   # [e, c*d]
        indtT = np.ascontiguousarray(
            ind_ced.transpose(2, 0, 1)).reshape(128, NCH * 128)   # [d, c*e]

        nfT = np.zeros((c.FV, c.NBP), np.float32)
        nfT[:, : c.NB] = nf[core * c.NB: (core + 1) * c.NB].T
        gidc = np.full(c.NBP, -1.0, np.float32)
        gidc[: c.NB] = gid[core * c.NB: (core + 1) * c.NB].astype(np.float32)

        odeg = np.zeros(c.NBP, np.float32)
        odeg[: c.NB] = outdeg[core * c.NB: (core + 1) * c.NB]
        ideg = np.zeros(c.NBP, np.float32)
        ideg[: c.NB] = indeg[core * c.NB: (core + 1) * c.NB]

        em = np.nonzero(owner == core)[0]
        efs = ef[em].sum(axis=0)                            # [41]
        pefsum = (efs[None, :] @ Wef2).reshape(c.L, c.ZF).astype(np.float32)

        m = {
            "srcrel": _wrap_idx16(srcrel.astype(np.int16)),
            "indt": indt.astype(BF_NP),
            "indtT": indtT.astype(BF_NP),
            "eft": eft.astype(BF_NP),
            "nfT": nfT,
            "gidc": gidc.reshape(-1, 128).T.copy(),
            "odeg": odeg.reshape(-1, 128).T.copy().astype(BF_NP),
            "ideg": ideg.reshape(-1, 128).T.copy().astype(BF_NP),
            "pefsum": pefsum.reshape(1, -1),
        }
        in_maps.append(m)

    shared = {
        "W_emb": np.asarray(inputs["W_emb"], np.float32),
        "g_emb": np.asarray(inputs["g_emb"], np.float32).reshape(1, E),
        "be_emb": np.asarray(inputs["be_emb"], np.float32).reshape(1, E),
        "Wsd2": Wsd2.astype(BF_NP),
        "Wef2": Wef2.astype(BF_NP),
        "gm": np.asarray(inputs["gm"], np.float32),
        "bem": np.asarray(inputs["bem"], np.float32),
        "gs": np.asarray(inputs["gs"], np.float32),
        "bes": np.asarray(inputs["bes"], np.float32),
        "gn": np.asarray(inputs["gn"], np.float32),
        "ben": np.asarray(inputs["ben"], np.float32),
        "Wf0": np.asarray(inputs["Wf0"], np.float32),
        "gf0": np.asarray(inputs["gf0"], np.float32).reshape(-1, 1),
        "bef0": np.asarray(inputs["bef0"], np.float32).reshape(-1, 1),
        "Wf1": np.asarray(inputs["Wf1"], np.float32),
        "gf1": np.asarray(inputs["gf1"], np.float32).reshape(-1, 1),
        "bef1": np.asarray(inputs["bef1"], np.float32).reshape(-1, 1),
        "Wt": np.asarray(inputs["Wt"], np.float32),
        "bt": np.asarray(inputs["bt"], np.float32).reshape(1, 1),
    }
    for m in in_maps:
        m.update(shared)
    return in_maps, EP


# ------------------------------------------------------------- kernel build
def build(cfg, EP):
    c = cfg
    EPT = EP[0] + EP[1]
    NCH = EPT // 128
    AF = mybir.ActivationFunctionType
    OP = mybir.AluOpType
    blkid = c.blkid
    assert blkid is not None and len(blkid) == NCH

    # static run structure: run = maximal chunk range with same (bucket, blk)
    nch0 = EP[0] // 128
    run_first = [False] * NCH
    run_last = [False] * NCH
    for ch in range(NCH):
        b = ch >= nch0
        pb = (ch - 1) >= nch0
        if ch == 0 or b != pb or blkid[ch] != blkid[ch - 1]:
            run_first[ch] = True
        if ch == NCH - 1 or (ch + 1 >= nch0) != b or blkid[ch + 1] != blkid[ch]:
            run_last[ch] = True

    nc = bacc.Bacc("TRN2", target_bir_lowering=False, debug=False,
                   enable_asserts=False, num_devices=c.NC, num_swdge_queues=4)

    def din(name, shape, dt=FP):
        return nc.dram_tensor(name, shape, dt, kind="ExternalInput")

    t_srcrel = din("srcrel", [128, EPT // 16], mybir.dt.int16)
    t_indt = din("indt", [128, NCH * 128], BF)
    t_indtT = din("indtT", [128, NCH * 128], BF)
    t_eft = din("eft", [c.FE, EPT], BF)
    t_nfT = din("nfT", [c.FV, c.NBP])
    t_gidc = din("gidc", [128, c.NBLK])
    t_odeg = din("odeg", [128, c.NBLK], BF)
    t_ideg = din("ideg", [128, c.NBLK], BF)
    t_pefsum = din("pefsum", [1, c.L * c.ZF])
    t_Wemb = din("W_emb", [c.FV, c.E])
    t_gemb = din("g_emb", [1, c.E])
    t_beemb = din("be_emb", [1, c.E])
    t_Wsd2 = din("Wsd2", [c.L, c.E, 2 * c.ZF], BF)
    t_Wef2 = din("Wef2", [c.L, c.FE, c.ZF], BF)
    t_gm = din("gm", [c.L, c.E])
    t_bem = din("bem", [c.L, c.E])
    t_gs = din("gs", [c.L, c.E])
    t_bes = din("bes", [c.L, c.E])
    t_gn = din("gn", [c.L, c.E])
    t_ben = din("ben", [c.L, c.E])
    t_Wf0 = din("Wf0", [c.E, c.FC0])
    t_gf0 = din("gf0", [c.FC0, 1])
    t_bef0 = din("bef0", [c.FC0, 1])
    t_Wf1 = din("Wf1", [c.FC0, c.FC1])
    t_gf1 = din("gf1", [c.FC1, 1])
    t_bef1 = din("bef1", [c.FC1, 1])
    t_Wt = din("Wt", [c.E, 1])
    t_bt = din("bt", [1, 1])
    t_out = nc.dram_tensor("out", [1, c.NG], FP, kind="ExternalOutput")

    RG = [list(range(c.NC))]

    with tile.TileContext(nc) as tc, ExitStack() as es:
        dram = es.enter_context(tc.tile_pool(name="dram", bufs=1, space="DRAM"))
        zbuf = dram.tile([128, NCH, c.ZF], BF)
        est_in = [dram.tile([1, 2 * c.ZF], FP, name=f"est_in{i}") for i in range(c.L)]
        est_out = [dram.tile([1, 2 * c.ZF], FP, addr_space="Shared", name=f"est_out{i}")
                   for i in range(c.L)]
        nst_in = [dram.tile([1, 2 * c.E], FP, name=f"nst_in{i}") for i in range(c.L + 1)]
        nst_out = [dram.tile([1, 2 * c.E], FP, addr_space="Shared", name=f"nst_out{i}")
                   for i in range(c.L + 1)]
        agin_l = [dram.tile([c.NBP, c.ZF], BF, name=f"agin{i}") for i in range(c.L)]
        agout_l = [dram.tile([c.NT, c.ZF], BF, addr_space="Shared", name=f"agout{i}")
                   for i in range(c.L)]
        ro_in = dram.tile([c.E + 1, c.NG], FP)
        ro_out = dram.tile([c.E + 1, c.NG], FP, addr_space="Shared")

        konst = es.enter_context(tc.tile_pool(name="konst", bufs=1))
        iotaF = konst.tile([128, 256], FP)
        identF = konst.tile([128, 128], FP)
        ones_row = konst.tile([1, 128], FP)
        ones_col = konst.tile([128, 1], FP)
        epsT = konst.tile([1, 1], FP)
        epsC = konst.tile([128, 1], FP)
        padmask = konst.tile([128, 1], FP)
        srcrel_sb = konst.tile([128, EPT // 16], mybir.dt.int16)
        nc.sync.dma_start(srcrel_sb[:], t_srcrel[:])
        with tc.tile_pool(name="ksetup", bufs=1) as ks:
            ii = ks.tile([128, 256], mybir.dt.int32)
            nc.gpsimd.iota(ii[:], pattern=[[1, 256]], base=0, channel_multiplier=0)
            nc.vector.tensor_copy(iotaF[:], ii[:])
            ip = ks.tile([128, 1], mybir.dt.int32)
            nc.gpsimd.iota(ip[:], pattern=[[1, 1]], base=0, channel_multiplier=1)
            ipf = ks.tile([128, 1], FP)
            nc.vector.tensor_copy(ipf[:], ip[:])
            nc.vector.tensor_scalar(identF[:], iotaF[:, :128], ipf[:], None, OP.is_equal)
            nc.vector.tensor_scalar(padmask[:], ipf[:], float(c.NB % 128), None, OP.is_lt)
        nc.vector.memset(ones_row[:], 1.0)
        nc.vector.memset(ones_col[:], 1.0)
        nc.vector.memset(epsT[:], c.EPS)
        nc.vector.memset(epsC[:], c.EPS)

        state = es.enter_context(tc.tile_pool(name="state", bufs=1))
        v_sb = state.tile([128, c.NBLK, c.E], FP)
        agg_sb = state.tile([128, c.NBLK, c.E], FP)
        adst_sb = state.tile([128, c.NBLK, c.ZF], BF)
        gid_sb = state.tile([128, c.NBLK], FP)
        odeg_sb = state.tile([128, c.NBLK], BF)
        ideg_sb = state.tile([128, c.NBLK], BF)
        pef_sb = state.tile([1, c.L * c.ZF], FP)
        nc.sync.dma_start(gid_sb[:], t_gidc[:])
        nc.sync.dma_start(odeg_sb[:], t_odeg[:])
        nc.sync.dma_start(ideg_sb[:], t_ideg[:])
        nc.sync.dma_start(pef_sb[:], t_pefsum[:])

        wts = es.enter_context(tc.tile_pool(name="wts", bufs=1))
        Wsd2_sb = wts.tile([c.E, c.L * 2 * c.ZF], BF)
        Wef2_sb = wts.tile([c.FE, c.L * c.ZF], BF)
        for l in range(c.L):
            nc.sync.dma_start(Wsd2_sb[:, l * 2 * c.ZF:(l + 1) * 2 * c.ZF], t_Wsd2[l])
            nc.sync.dma_start(Wef2_sb[:, l * c.ZF:(l + 1) * c.ZF], t_Wef2[l])

        # sigmoid(x) -> out, via one act table: sig = exp(-softplus(-x))
        def sigmoid_ops(pool, out, x, shape, nm):
            t1 = pool.tile(shape, FP, name=f"sgA{nm}", tag=f"sgA{nm}")
            nc.scalar.activation(t1[:], x, AF.Abs)
            nc.scalar.activation(t1[:], t1[:], AF.Exp, scale=-1.0)
            nc.any.tensor_scalar_add(t1[:], t1[:], 1.0)
            nc.scalar.activation(t1[:], t1[:], AF.Ln)
            t2 = pool.tile(shape, FP, name=f"sgB{nm}", tag=f"sgB{nm}")
            nc.vector.tensor_scalar(t2[:], x, 0.0, -1.0, OP.min, OP.mult)
            nc.any.tensor_add(t1[:], t1[:], t2[:])
            nc.scalar.activation(out, t1[:], AF.Exp, scale=-1.0)

        # softplus(x) -> out = ln(1+exp(-|x|)) + relu(x)
        def softplus_ops(pool, out, x, shape, nm):
            t1 = pool.tile(shape, FP, name=f"spA{nm}", tag=f"spA{nm}")
            nc.scalar.activation(t1[:], x, AF.Abs)
            nc.scalar.activation(t1[:], t1[:], AF.Exp, scale=-1.0)
            nc.any.tensor_scalar_add(t1[:], t1[:], 1.0)
            nc.scalar.activation(t1[:], t1[:], AF.Ln)
            t2 = pool.tile(shape, FP, name=f"spB{nm}", tag=f"spB{nm}")
            nc.scalar.activation(t2[:], x, AF.Relu)
            nc.any.tensor_add(out, t1[:], t2[:])

        def bn_fold(pool, sums, F, count, g_ap, be_ap):
            st = pool.tile([1, 2 * F], FP, name=f"bnf{nc.next_id()}")
            mean = pool.tile([1, F], FP, name=f"bnm{nc.next_id()}")
            var = pool.tile([1, F], FP, name=f"bnv{nc.next_id()}")
            nc.scalar.mul(mean[:], sums[:, 0:F], 1.0 / count)
            nc.scalar.mul(var[:], sums[:, F:2 * F], 1.0 / count)
            m2 = pool.tile([1, F], FP, name=f"bn2{nc.next_id()}")
            nc.vector.tensor_mul(m2[:], mean[:], mean[:])
            nc.vector.tensor_sub(var[:], var[:], m2[:])
            nc.scalar.activation(var[:], var[:], AF.Ln, bias=epsT[0:1, 0:1])
            nc.scalar.activation(var[:], var[:], AF.Exp, scale=-0.5)
            nc.vector.tensor_mul(st[:, 0:F], g_ap, var[:])
            nc.vector.tensor_mul(mean[:], mean[:], st[:, 0:F])
            nc.vector.tensor_sub(st[:, F:2 * F], be_ap, mean[:])
            return st

        def bcast_row(pool, psum_pool, row_ap, W, name):
            ps = psum_pool.tile([128, W], FP, name=f"ps{name}")
            nc.tensor.matmul(ps[:], ones_row[:, :], row_ap, start=True, stop=True)
            sb = pool.tile([128, W], FP, name=name)
            nc.scalar.copy(sb[:], ps[:])
            return sb

        def zero_vpad():
            # zero pad-node rows of the last block (per-partition mask multiply)
            cb = c.NB // 128
            nc.vector.tensor_scalar(v_sb[:, cb, :], v_sb[:, cb, :],
                                    padmask[:], None, OP.mult)

        # ---------------------------------------------------- embedding
        with tc.tile_pool(name="emb", bufs=1) as emb, \
             tc.tile_pool(name="embw", bufs=2) as embw, \
             tc.tile_pool(name="embp", bufs=2, space="PSUM") as embp, \
             tc.tile_pool(name="embs", bufs=1, space="PSUM") as embs:
            nfT_sb = emb.tile([c.FV, c.NBP], FP)
            nc.sync.dma_start(nfT_sb[:], t_nfT[:])
            Wemb_sb = emb.tile([c.FV, c.E], FP)
            nc.sync.dma_start(Wemb_sb[:], t_Wemb[:])
            z0 = emb.tile([128, c.NBLK, c.E], FP)
            ssum = embs.tile([1, c.E], FP)
            ssq = embs.tile([1, c.E], FP)
            for ch in range(c.NBLK):
                ps = embp.tile([128, c.E], FP, name="embz")
                nc.tensor.matmul(ps[:], nfT_sb[:, ch * 128:(ch + 1) * 128],
                                 Wemb_sb[:], start=True, stop=True)
                nc.scalar.copy(z0[:, ch, :], ps[:])
                sq = embw.tile([128, c.E], FP, name="embsq")
                nc.vector.tensor_mul(sq[:], z0[:, ch, :], z0[:, ch, :])
                nc.tensor.matmul(ssum[:], ones_col[:, :], z0[:, ch, :],
                                 start=(ch == 0), stop=(ch == c.NBLK - 1))
                nc.tensor.matmul(ssq[:], ones_col[:, :], sq[:],
                                 start=(ch == 0), stop=(ch == c.NBLK - 1))
            stat = emb.tile([1, 2 * c.E], FP)
            nc.vector.tensor_copy(stat[:, 0:c.E], ssum[:])
            nc.vector.tensor_copy(stat[:, c.E:], ssq[:])
            nc.sync.dma_start(nst_in[c.L][:], stat[:])
            nc.gpsimd.collective_compute(
                "AllReduce", OP.add, replica_groups=RG,
                ins=[nst_in[c.L].opt()], outs=[nst_out[c.L].opt()])
            rstat = emb.tile([1, 2 * c.E], FP)
            nc.sync.dma_start(rstat[:], nst_out[c.L][:])
            gemb_sb = emb.tile([1, c.E], FP)
            beemb_sb = emb.tile([1, c.E], FP)
            nc.sync.dma_start(gemb_sb[:], t_gemb[:])
            nc.sync.dma_start(beemb_sb[:], t_beemb[:])
            st = bn_fold(emb, rstat, c.E, c.N, gemb_sb[:], beemb_sb[:])
            stb = bcast_row(emb, embp, st[:], 2 * c.E, "embst")
            for ch in range(c.NBLK):
                u = embw.tile([128, c.E], FP, name="embu")
                nc.vector.tensor_mul(u[:], z0[:, ch, :], stb[:, 0:c.E])
                nc.vector.tensor_add(u[:], u[:], stb[:, c.E:])
                sg = embw.tile([128, c.E], FP, name="embsg")
                sigmoid_ops(embw, sg[:], u[:], [128, c.E], "emb")
                nc.vector.tensor_mul(v_sb[:, ch, :], u[:], sg[:])
            zero_vpad()

        # ---------------------------------------------------- conv layers
        gq = 0
        for l in range(c.L):
            # ---- phase A: projection tables [A_src | A_dst] per block
            with tc.tile_pool(name="phS", bufs=1, space="PSUM") as phs, \
                 tc.tile_pool(name="phA", bufs=3) as pa:
                s1 = phs.tile([1, c.ZF], FP)
                s2 = phs.tile([1, c.ZF], FP)
                asrc_sb = pa.tile([128, c.NBLK, c.ZF], BF, bufs=1)
                with tc.tile_pool(name="phAp", bufs=2, space="PSUM") as pap, \
                     tc.tile_pool(name="phAo", bufs=2, space="PSUM") as pao:
                    for ch in range(c.NBLK):
                        vt_ps = pap.tile([c.E, 128], FP, name="vtps")
                        nc.tensor.transpose(vt_ps[:], v_sb[:, ch, :], identF[:])
                        vt = pa.tile([c.E, 128], BF, name="vt")
                        nc.vector.tensor_copy(vt[:], vt_ps[:])
                        ao = pao.tile([128, 2 * c.ZF], FP, name="ao")
                        nc.tensor.matmul(ao[:], vt[:],
                                         Wsd2_sb[:, l * 2 * c.ZF:(l + 1) * 2 * c.ZF],
                                         start=True, stop=True)
                        nc.vector.tensor_copy(asrc_sb[:, ch, :], ao[:, 0:c.ZF])
                        nc.vector.tensor_copy(adst_sb[:, ch, :], ao[:, c.ZF:])
                        nc.tensor.matmul(s1[:], odeg_sb[:, ch:ch + 1],
                                         asrc_sb[:, ch, :],
                                         start=(ch == 0), stop=(ch == c.NBLK - 1))
                        nc.tensor.matmul(s2[:], ideg_sb[:, ch:ch + 1],
                                         adst_sb[:, ch, :],
                                         start=(ch == 0), stop=(ch == c.NBLK - 1))
                nc.sync.dma_start(
                    agin_l[l][:].rearrange("(b p) f -> p b f", p=128), asrc_sb[:])

                nc.gpsimd.collective_compute(
                    "AllGather", OP.bypass, replica_groups=RG,
                    ins=[agin_l[l].opt()], outs=[agout_l[l].opt()])

                # ---- pass 1: z build + spill + sum(z^2)
                with tc.tile_pool(name="p1g", bufs=3) as pg, \
                     tc.tile_pool(name="p1m", bufs=2) as pm, \
                     tc.tile_pool(name="p1z", bufs=2) as pz, \
                     tc.tile_pool(name="p1acc", bufs=1) as pacc, \
                     tc.tile_pool(name="p1zp", bufs=2, space="PSUM") as pzp:
                    acc_q = pacc.tile([128, c.GS, c.ZF], FP)
                    nc.vector.memset(acc_q[:], 0.0)
                    for b in (0, 1):
                        nchb = EP[b] // 128
                        base_ch = (0 if b == 0 else nch0)
                        for g0 in range(0, nchb, c.GS):
                            gs = min(c.GS, nchb - g0)
                            ni = gs * 128
                            coff = base_ch + g0
                            gsr_t = pg.tile([128, c.GS, c.ZF], BF, name="gsrc")
                            nc.gpsimd.dma_gather(
                                gsr_t[:, :gs, :],
                                agout_l[l][b * c.HALF:(b + 1) * c.HALF, :],
                                srcrel_sb[:, coff * 8:coff * 8 + gs * 8],
                                num_idxs=ni, num_idxs_reg=ni,
                                elem_size=c.ZF, queue_num=gq % 4,
                                single_packet=False)
                            gq += 1
                            mde_t = pm.tile([128, c.GS, 128], BF, name="mde")
                            nc.sync.dma_start(
                                mde_t[:, :gs, :],
                                t_indtT[:, coff * 128:coff * 128 + ni])
                            ef_t = pm.tile([c.FE, c.GS * 128], BF, name="eft")
                            nc.sync.dma_start(ef_t[:, :ni],
                                              t_eft[:, coff * 128:coff * 128 + ni])
                            z_t = pz.tile([128, c.GS, c.ZF], BF, name="zt")
                            for s0 in range(0, gs, 8):
                                sub = min(8, gs - s0)
                                zps = pzp.tile([128, 8, c.ZF], FP, name="zps")
                                for j in range(s0, s0 + sub):
                                    jj = j - s0
                                    bk = blkid[coff + j]
                                    nc.tensor.matmul(
                                        zps[:, jj, :],
                                        ef_t[:, j * 128:(j + 1) * 128],
                                        Wef2_sb[:, l * c.ZF:(l + 1) * c.ZF],
                                        start=True, stop=False)
                                    nc.tensor.matmul(
                                        zps[:, jj, :], mde_t[:, j, :],
                                        adst_sb[:, bk, :],
                                        start=False, stop=True)
                                nc.vector.tensor_tensor(
                                    z_t[:, s0:s0 + sub, :], zps[:, :sub, :],
                                    gsr_t[:, s0:s0 + sub, :], OP.add)
                            nc.sync.dma_start(zbuf[:, coff:coff + gs, :],
                                              z_t[:, :gs, :])
                            sq_t = pz.tile([128, c.GS, c.ZF], BF, name="sqt")
                            nc.scalar.square(sq_t[:, :gs, :], z_t[:, :gs, :])
                            nc.vector.tensor_tensor(
                                acc_q[:, :gs, :], acc_q[:, :gs, :],
                                sq_t[:, :gs, :], OP.add)
                    with tc.tile_pool(name="p1st", bufs=1) as pst, \
                         tc.tile_pool(name="p1sp", bufs=1, space="PSUM") as psp:
                        red_q = pst.tile([128, c.ZF], FP)
                        nc.vector.tensor_reduce(
                            red_q[:], acc_q[:].rearrange("p g f -> p f g"),
                            mybir.AxisListType.X, OP.add)
                        psq = psp.tile([1, c.ZF], FP, name="psq")
                        nc.tensor.matmul(psq[:], ones_col[:, :], red_q[:],
                                         start=True, stop=True)
                        stat = pst.tile([1, 2 * c.ZF], FP)
                        nc.vector.tensor_tensor(
                            stat[:, :c.ZF], s1[:],
                            pef_sb[:, l * c.ZF:(l + 1) * c.ZF], OP.add)
                        nc.vector.tensor_tensor(stat[:, :c.ZF], stat[:, :c.ZF],
                                                s2[:], OP.add)
                        nc.vector.tensor_copy(stat[:, c.ZF:], psq[:])
                        nc.sync.dma_start(est_in[l][:], stat[:])

            nc.gpsimd.collective_compute(
                "AllReduce", OP.add, replica_groups=RG,
                ins=[est_in[l].opt()], outs=[est_out[l].opt()])

            # ---- pass 2: activations + scatter
            with tc.tile_pool(name="p2", bufs=1) as p2, \
                 tc.tile_pool(name="p2z", bufs=2) as p2z, \
                 tc.tile_pool(name="p2h", bufs=2) as p2h, \
                 tc.tile_pool(name="p2ap", bufs=4, space="PSUM") as p2ap, \
                 tc.tile_pool(name="p2bp", bufs=1, space="PSUM") as p2bp:
                rstat = p2.tile([1, 2 * c.ZF], FP)
                nc.sync.dma_start(rstat[:], est_out[l][:])
                gms = p2.tile([1, 2 * c.E], FP)
                nc.sync.dma_start(gms[:, :c.E], t_gm[l:l + 1, :])
                nc.sync.dma_start(gms[:, c.E:], t_gs[l:l + 1, :])
                bms = p2.tile([1, 2 * c.E], FP)
                nc.sync.dma_start(bms[:, :c.E], t_bem[l:l + 1, :])
                nc.sync.dma_start(bms[:, c.E:], t_bes[l:l + 1, :])
                st = bn_fold(p2, rstat, c.ZF, c.M, gms[:], bms[:])
                stb = bcast_row(p2, p2bp, st[:], 2 * c.ZF, "edgest")
                s_g = p2.tile([128, c.GS, c.ZF], BF)
                t_g = p2.tile([128, c.GS, c.ZF], BF)
                for j in range(c.GS):
                    nc.vector.tensor_copy(s_g[:, j, :], stb[:, 0:c.ZF])
                    nc.vector.tensor_copy(t_g[:, j, :], stb[:, c.ZF:])
                nc.vector.memset(agg_sb[:], 0.0)
                aps = None
                for b in (0, 1):
                    nchb = EP[b] // 128
                    base_ch = (0 if b == 0 else nch0)
                    for g0 in range(0, nchb, c.GS):
                        gs = min(c.GS, nchb - g0)
                        ni = gs * 128
                        coff = base_ch + g0
                        z_t = p2z.tile([128, c.GS, c.ZF], BF, name="z2t")
                        nc.sync.dma_start(z_t[:, :gs, :],
                                          zbuf[:, coff:coff + gs, :])
                        ind_t = p2z.tile([128, c.GS, 128], BF, name="indt")
                        nc.sync.dma_start(
                            ind_t[:, :gs, :],
                            t_indt[:, coff * 128:coff * 128 + ni])
                        u = p2z.tile([128, c.GS, c.ZF], BF, name="u")
                        nc.vector.tensor_tensor(u[:, :gs, :], z_t[:, :gs, :],
                                                s_g[:, :gs, :], OP.mult)
                        nc.vector.tensor_tensor(u[:, :gs, :], u[:, :gs, :],
                                                t_g[:, :gs, :], OP.add)
                        # core = ln(1 + exp(-|u|)) on both halves at once
                        core = p2z.tile([128, c.GS, c.ZF], BF, name="core")
                        nc.scalar.activation(core[:, :gs, :], u[:, :gs, :], AF.Abs)
                        nc.scalar.activation(core[:, :gs, :], core[:, :gs, :],
                                             AF.Exp, scale=-1.0)
                        nc.scalar.activation(core[:, :gs, :], core[:, :gs, :],
                                             AF.Ln, bias=ones_col[:, :])
                        # softplus(us) = core_s + relu(us)
                        sp = p2h.tile([128, c.GS, c.E], BF, name="sp")
                        nc.vector.tensor_scalar_max(sp[:, :gs, :],
                                                    u[:, :gs, c.E:], 0.0)
                        nc.vector.tensor_tensor(sp[:, :gs, :], sp[:, :gs, :],
                                                core[:, :gs, c.E:], OP.add)
                        # sigmoid(um) = exp(-(core_m + relu(-um)))
                        sg = p2h.tile([128, c.GS, c.E], BF, name="sg")
                        nc.vector.tensor_scalar(sg[:, :gs, :], u[:, :gs, 0:c.E],
                                                0.0, -1.0, OP.min, OP.mult)
                        nc.vector.tensor_tensor(sg[:, :gs, :], sg[:, :gs, :],
                                                core[:, :gs, 0:c.E], OP.add)
                        nc.scalar.activation(sg[:, :gs, :], sg[:, :gs, :],
                                             AF.Exp, scale=-1.0)
                        h = p2h.tile([128, c.GS, c.E], BF, name="h")
                        nc.vector.tensor_tensor(h[:, :gs, :], sg[:, :gs, :],
                                                sp[:, :gs, :], OP.mult)
                        for j in range(gs):
                            ch = coff + j
                            bk = blkid[ch]
                            if run_first[ch]:
                                aps = p2ap.tile([128, c.E], FP, name="psagg")
                            nc.tensor.matmul(aps[:], ind_t[:, j, :], h[:, j, :],
                                             start=run_first[ch],
                                             stop=run_last[ch])
                            if run_last[ch]:
                                nc.vector.tensor_tensor(
                                    agg_sb[:, bk, :], agg_sb[:, bk, :],
                                    aps[:], OP.add)

            # ---- node BN + update
            with tc.tile_pool(name="nod", bufs=1) as nod, \
                 tc.tile_pool(name="nodw", bufs=2) as nodw, \
                 tc.tile_pool(name="nodp", bufs=2, space="PSUM") as nodp, \
                 tc.tile_pool(name="nods", bufs=1, space="PSUM") as nods:
                nsum = nods.tile([1, c.E], FP)
                nssq = nods.tile([1, c.E], FP)
                for ch in range(c.NBLK):
                    sq = nodw.tile([128, c.E], FP, name="nsq")
                    nc.vector.tensor_mul(sq[:], agg_sb[:, ch, :], agg_sb[:, ch, :])
                    nc.tensor.matmul(nsum[:], ones_col[:, :], agg_sb[:, ch, :],
                                     start=(ch == 0), stop=(ch == c.NBLK - 1))
                    nc.tensor.matmul(nssq[:], ones_col[:, :], sq[:],
                                     start=(ch == 0), stop=(ch == c.NBLK - 1))
                stat = nod.tile([1, 2 * c.E], FP)
                nc.vector.tensor_copy(stat[:, :c.E], nsum[:])
                nc.vector.tensor_copy(stat[:, c.E:], nssq[:])
                nc.sync.dma_start(nst_in[l][:], stat[:])
                nc.gpsimd.collective_compute(
                    "AllReduce", OP.add, replica_groups=RG,
                    ins=[nst_in[l].opt()], outs=[nst_out[l].opt()])
                rstat = nod.tile([1, 2 * c.E], FP)
                nc.sync.dma_start(rstat[:], nst_out[l][:])
                gn_sb = nod.tile([1, c.E], FP)
                ben_sb = nod.tile([1, c.E], FP)
                nc.sync.dma_start(gn_sb[:], t_gn[l:l + 1, :])
                nc.sync.dma_start(ben_sb[:], t_ben[l:l + 1, :])
                st = bn_fold(nod, rstat, c.E, c.N, gn_sb[:], ben_sb[:])
                stb = bcast_row(nod, nodp, st[:], 2 * c.E, "nodst")
                for ch in range(c.NBLK):
                    u = nodw.tile([128, c.E], FP, name="nu")
                    nc.vector.tensor_mul(u[:], agg_sb[:, ch, :], stb[:, 0:c.E])
                    nc.vector.tensor_add(u[:], u[:], stb[:, c.E:])
                    nc.vector.tensor_add(u[:], u[:], v_sb[:, ch, :])
                    softplus_ops(nodw, v_sb[:, ch, :], u[:], [128, c.E], "nod")
                zero_vpad()

        # ---------------------------------------------------- readout
        with tc.tile_pool(name="ro", bufs=1) as ro, \
             tc.tile_pool(name="row", bufs=2) as row, \
             tc.tile_pool(name="rop", bufs=1, space="PSUM") as rop, \
             tc.tile_pool(name="ros", bufs=1, space="PSUM") as ros:
            psums = ros.tile([c.E, c.NG], FP)
            pcnt = ros.tile([1, c.NG], FP)
            for ch in range(c.NBLK):
                gind = row.tile([128, c.NG], FP, name="gind")
                nc.vector.tensor_scalar(gind[:], iotaF[:, :c.NG],
                                        gid_sb[:, ch:ch + 1], None, OP.is_equal)
                nc.tensor.matmul(psums[:], v_sb[:, ch, :], gind[:],
                                 start=(ch == 0), stop=(ch == c.NBLK - 1))
                nc.tensor.matmul(pcnt[:], ones_col[:, :], gind[:],
                                 start=(ch == 0), stop=(ch == c.NBLK - 1))
            acc = ro.tile([c.E + 1, c.NG], FP)
            nc.scalar.copy(acc[0:c.E, :], psums[:])
            nc.scalar.copy(acc[c.E:c.E + 1, :], pcnt[:])
            nc.sync.dma_start(ro_in[:], acc[:])
            nc.gpsimd.collective_compute(
                "AllReduce", OP.add, replica_groups=RG,
                ins=[ro_in.opt()], outs=[ro_out.opt()])
            racc = ro.tile([c.E + 1, c.NG], FP)
            nc.sync.dma_start(racc[:], ro_out[:])
            cnt = ro.tile([1, c.NG], FP)
            nc.vector.tensor_scalar_max(cnt[:], racc[c.E:c.E + 1, :], 1.0)
            nc.vector.reciprocal(cnt[:], cnt[:])
            rcb_ps = rop.tile([c.E, c.NG], FP, name="rcb", tag="rosc")
            nc.tensor.matmul(rcb_ps[:], ones_row[:, 0:c.E], cnt[:], start=True, stop=True)
            vs = ro.tile([c.E, c.NG], FP)
            nc.vector.tensor_tensor(vs[:], racc[0:c.E, :], rcb_ps[:], OP.mult)

            def fc_bn_silu(pool, psum_pool, x_sb, W_ap, K, Fo, g_t, be_t, nm):
                ps = psum_pool.tile([Fo, c.NG], FP, name=f"fc{nm}", tag="rosc")
                W_sb = pool.tile([K, Fo], FP, name=f"W{nm}")
                nc.sync.dma_start(W_sb[:], W_ap)
                nc.tensor.matmul(ps[:], W_sb[:], x_sb[:], start=True, stop=True)
                g_sb = pool.tile([Fo, 1], FP, name=f"g{nm}")
                be_sb = pool.tile([Fo, 1], FP, name=f"be{nm}")
                nc.sync.dma_start(g_sb[:], g_t[:])
                nc.sync.dma_start(be_sb[:], be_t[:])
                x_sbc = pool.tile([Fo, c.NG], FP, name=f"x{nm}")
                nc.scalar.copy(x_sbc[:], ps[:])
                sums = pool.tile([Fo, 1], FP, name=f"su{nm}")
                nc.vector.tensor_reduce(sums[:], x_sbc[:], mybir.AxisListType.X, OP.add)
                sq = pool.tile([Fo, c.NG], FP, name=f"sq{nm}")
                nc.vector.tensor_mul(sq[:], x_sbc[:], x_sbc[:])
                ssq = pool.tile([Fo, 1], FP, name=f"sl{nm}")
                nc.vector.tensor_reduce(ssq[:], sq[:], mybir.AxisListType.X, OP.add)
                mean = pool.tile([Fo, 1], FP, name=f"mn{nm}")
                nc.scalar.mul(mean[:], sums[:], 1.0 / c.NG)
                var = pool.tile([Fo, 1], FP, name=f"vr{nm}")
                nc.scalar.mul(var[:], ssq[:], 1.0 / c.NG)
                m2 = pool.tile([Fo, 1], FP, name=f"m2{nm}")
                nc.vector.tensor_mul(m2[:], mean[:], mean[:])
                nc.vector.tensor_sub(var[:], var[:], m2[:])
                nc.scalar.activation(var[:], var[:], AF.Ln, bias=epsC[0:Fo, :])
                nc.scalar.activation(var[:], var[:], AF.Exp, scale=-0.5)
                s_col = pool.tile([Fo, 1], FP, name=f"sc{nm}")
                nc.vector.tensor_mul(s_col[:], g_sb[:], var[:])
                t_col = pool.tile([Fo, 1], FP, name=f"tc{nm}")
                nc.vector.tensor_mul(t_col[:], mean[:], s_col[:])
                nc.vector.tensor_sub(t_col[:], be_sb[:], t_col[:])
                u = pool.tile([Fo, c.NG], FP, name=f"u{nm}")
                nc.scalar.activation(u[:], x_sbc[:], AF.Identity,
                                     bias=t_col[:], scale=s_col[:])
                sg2 = pool.tile([Fo, c.NG], FP, name=f"sg{nm}")
                sigmoid_ops(pool, sg2[:], u[:], [Fo, c.NG], f"fc{nm}")
                out = pool.tile([Fo, c.NG], FP, name=f"o{nm}")
                nc.vector.tensor_mul(out[:], u[:], sg2[:])
                return out

            z1 = fc_bn_silu(ro, rop, vs, t_Wf0[:], c.E, c.FC0, t_gf0, t_bef0, "0")
            z2 = fc_bn_silu(ro, rop, z1, t_Wf1[:], c.FC0, c.FC1, t_gf1, t_bef1, "1")
            Wt_sb = ro.tile([c.E, 1], FP)
            nc.sync.dma_start(Wt_sb[:], t_Wt[:])
            hd = rop.tile([1, c.NG], FP, name="hd", tag="rosc")
            nc.tensor.matmul(hd[:], Wt_sb[:], z2[:], start=True, stop=True)
            bt_sb = ro.tile([1, 1], FP)
            nc.sync.dma_start(bt_sb[:], t_bt[:])
            res = ro.tile([1, c.NG], FP)
            nc.vector.tensor_scalar(res[:], hd[:], bt_sb[0:1, 0:1], None, OP.add)
            nc.sync.dma_start(t_out[:], res[:])

    nc.compile()
    return nc


# ------------------------------------------------------------------ driver
_CACHE = {}


def kernel(**inputs):
    cfg = Cfg(int(inputs["node_feats"].shape[0]),
              int(inputs["src"].shape[0]), 256)
    in_maps, EP = preprocess(inputs, cfg)
    key = (cfg.N, cfg.M, tuple(EP), cfg.blkid)
    if key not in _CACHE:
        _CACHE[key] = build(cfg, EP)
    nc = _CACHE[key]
    res = bass_utils.run_bass_kernel_spmd(
        nc, in_maps, core_ids=list(range(cfg.NC)), trace=False)
    out = np.asarray(res.results[0]["out"], np.float32)
    return out.reshape(cfg.NG, 1)
